# revision 20
# baseline (speedup 1.0000x reference)
"""ArcFace-style loss kernel for Trainium2 (8 NeuronCores).

Strategy
--------
The only heavy tensor is ``weight`` [200000, 192] (153.6 MB f32).  The loss
needs, per (b, m) embedding row:

  * ``sum_full[b,m] = sum_c exp(SCALE * cos[b,m,c] - SCALE)``   (fixed shift:
    cos <= 1 always, so SCALE is a valid stable shift — identical math to the
    reference's row-max shift),
  * the cosine at the 4 ground-truth label columns (tiny: 128 rows of W).

Device (per core, classes sharded 8-way -> 25000 classes/core):
  DMA pre-normalized, pre-transposed W^T slice [192, 25000] -> SBUF,
  matmul (xn^T stationary [96,128] x2 K-chunks, W^T moving) -> PSUM [128, N],
  ScalarE activation Exp(30*x - 30) with accum_out -> partial row sums,
  final DVE reduce -> [128, 1] partial logsumexp denominator per core.

Host: l2-normalize x and W (cheap marshalling passes), all-reduce the 8
partial sums, gather the 128 label rows of W for exact cos_l, then the
O(B*M*S) ArcFace + Hungarian + BCE epilogue in float64.
"""

import math
import os
from contextlib import ExitStack

import numpy as np

import concourse.bass as bass
import concourse.tile as tile
from concourse import bacc, mybir
from concourse.bass_utils import run_bass_kernel_spmd

# ---- problem constants (hardcoded per contract) ----
B, M, D, NC = 32, 4, 192, 200000
BM = B * M                       # 128 rows
N_CORES = 8
C_SH = NC // N_CORES             # 25000 classes per core
S_SPK = 4
SCALE = 30.0
MARGIN = 0.5
ETA, XI = 2.5, 5.0
COS_M = math.cos(MARGIN)
SIN_M = math.sin(MARGIN)
TH = math.cos(math.pi - MARGIN)
MM = math.sin(math.pi - MARGIN) * MARGIN
EPS = 1e-6

# ---- kernel tiling ----
PSUM_BANK = 512   # f32 elements per PSUM bank (matmul output may not cross)
BLK = 5000        # classes per W block (SBUF tile)
DMA_CHUNK = 1250  # classes per dma_start (subtile deps let matmuls start early)
K0 = 96           # D split 96+96 for the contraction

# matmul input dtype: "f32r" (full-rate fp32), "f32" (4x slower), "bf16"
DTYPE = os.environ.get("KERNEL_DTYPE", "bf16")

LAST_EXEC_NS = None
LAST_RESULTS = None

_CACHE = {}


def _mm_dt(name):
    return {
        "f32": mybir.dt.float32,
        "f32r": mybir.dt.float32r,
        "bf16": mybir.dt.bfloat16,
    }[name]


def _np_dt(name):
    if name == "bf16":
        import ml_dtypes

        return np.dtype(ml_dtypes.bfloat16)
    return np.dtype(np.float32)


def _build(dtype_name, c_sh=C_SH, blk=BLK):
    dt_in = _mm_dt(dtype_name)
    f32 = mybir.dt.float32
    AF = mybir.ActivationFunctionType

    nc = bacc.Bacc(
        "TRN2", target_bir_lowering=False, debug=False, num_devices=N_CORES
    )
    wt = nc.dram_tensor("wt", [D, c_sh], dt_in, kind="ExternalInput").ap()
    # x^T packed as [96, 256]: cols 0:128 = D rows 0:96, cols 128:256 = D rows
    # 96:192 — one DMA instead of two
    xt = nc.dram_tensor("xt", [K0, 2 * BM], dt_in, kind="ExternalInput").ap()
    out = nc.dram_tensor("out", [BM, 1], f32, kind="ExternalOutput").ap()

    assert c_sh % blk == 0
    n_blk = c_sh // blk
    ex_dt = mybir.dt.bfloat16 if dtype_name == "bf16" else f32

    # split a block into "supers" (one activation each); each super is a list
    # of matmul widths, every matmul bank-aligned inside the super's psum tile
    def _supers(width):
        sups = []
        rem = width
        while rem >= 2 * PSUM_BANK:
            sups.append([PSUM_BANK, PSUM_BANK])
            rem -= 2 * PSUM_BANK
        if rem > PSUM_BANK:
            sups.append([PSUM_BANK, rem - PSUM_BANK])
        elif rem > 0:
            sups.append([rem])
        return sups

    blk_supers = _supers(blk)
    n_super = n_blk * len(blk_supers)

    with tile.TileContext(nc) as tc, ExitStack() as ctx:
        xp = ctx.enter_context(tc.tile_pool(name="x", bufs=1))
        wp = ctx.enter_context(tc.tile_pool(name="w", bufs=3))
        pp = ctx.enter_context(tc.tile_pool(name="ps", bufs=3, space="PSUM"))
        ep = ctx.enter_context(tc.tile_pool(name="ex", bufs=3))
        accp = ctx.enter_context(tc.tile_pool(name="acc", bufs=1))

        xtile = xp.tile([K0, 2 * BM], dt_in, tag="xt")
        nc.sync.dma_start(xtile[:], xt[:, :])
        x0 = xtile[:, 0:BM]
        x1 = xtile[:, BM : 2 * BM]

        acc = accp.tile([BM, n_super], f32, tag="acc")
        bias_t = accp.tile([BM, 1], f32, tag="bias")
        nc.gpsimd.memset(bias_t[:], -SCALE)
        # dummy 1-elem Exp: pulls the ~2.7us activation-table load off the
        # critical path (overlaps the first W DMA)
        warm = accp.tile([BM, 1], f32, tag="warm")
        nc.scalar.activation(warm[:], bias_t[:], AF.Exp, bias=bias_t[:], scale=0.0)

        for b in range(n_blk):
            w0 = wp.tile([K0, blk], dt_in, tag="w0")
            w1 = wp.tile([D - K0, blk], dt_in, tag="w1")
            for c0 in range(0, blk, DMA_CHUNK):
                g = b * blk + c0
                cw = min(DMA_CHUNK, blk - c0)
                nc.sync.dma_start(w0[:, c0 : c0 + cw], wt[0:K0, g : g + cw])
                nc.sync.dma_start(w1[:, c0 : c0 + cw], wt[K0:D, g : g + cw])
            sup_off = 0
            for s, widths in enumerate(blk_supers):
                sup_w = sum(widths)
                # psum tile: one bank per matmul, activation reads only the
                # live columns [0:sup_w] (bank 1 starts at PSUM_BANK)
                ps_banks = len(widths)
                ps = pp.tile([BM, ps_banks * PSUM_BANK], f32, tag="ps")
                for t, w in enumerate(widths):
                    off = sup_off + t * PSUM_BANK
                    dst = ps[:, t * PSUM_BANK : t * PSUM_BANK + w]
                    nc.tensor.matmul(
                        dst, x0, w0[:, off : off + w], start=True, stop=False
                    )
                    nc.tensor.matmul(
                        dst, x1, w1[:, off : off + w], start=False, stop=True
                    )
                ex = ep.tile([BM, ps_banks * PSUM_BANK], ex_dt, tag="ex")
                j = b * len(blk_supers) + s
                last = b == n_blk - 1 and s == len(blk_supers) - 1
                if last:
                    # final super: ACT's fused accumulator shortens the tail
                    # (skips the DVE reduce hop on the critical path)
                    nc.scalar.activation(
                        ex[:, :sup_w],
                        ps[:, :sup_w],
                        AF.Exp,
                        bias=bias_t[:],
                        scale=SCALE,
                        accum_out=acc[:, j : j + 1],
                    )
                else:
                    nc.scalar.activation(
                        ex[:, :sup_w], ps[:, :sup_w], AF.Exp, bias=bias_t[:], scale=SCALE
                    )
                    nc.vector.tensor_reduce(
                        acc[:, j : j + 1],
                        ex[:, :sup_w],
                        axis=mybir.AxisListType.X,
                        op=mybir.AluOpType.add,
                    )
                sup_off += sup_w
        part = accp.tile([BM, 1], f32, tag="part")
        nc.vector.tensor_reduce(
            part[:], acc[:], axis=mybir.AxisListType.X, op=mybir.AluOpType.add
        )
        nc.sync.dma_start(out, part[:])

    nc.compile()
    return nc


def _get_nc(dtype_name):
    if dtype_name not in _CACHE:
        _CACHE[dtype_name] = _build(dtype_name)
    return _CACHE[dtype_name]


def _l2n(x, axis=-1):
    n = np.linalg.norm(x.astype(np.float32), axis=axis, keepdims=True)
    return x / np.maximum(n, 1e-12)


def _device_sumexp(xn, wn, dtype_name, trace=False):
    """Run the 8-core SPMD kernel. xn: [BM, D] f32 normalized rows;
    wn: [NC, D] f32 normalized rows. Returns sum_full [BM] f64."""
    global LAST_EXEC_NS, LAST_RESULTS
    np_dt = _np_dt(dtype_name)
    xT_full = xn.T.astype(np_dt)                           # [D, BM]
    xT = np.ascontiguousarray(
        np.concatenate([xT_full[0:96], xT_full[96:192]], axis=1)
    )                                                      # [96, 256] packed
    wT = np.ascontiguousarray(wn.T.astype(np_dt))          # [D, NC]
    in_maps = []
    for k in range(N_CORES):
        sl = wT[:, k * C_SH : (k + 1) * C_SH]
        in_maps.append({"wt": np.ascontiguousarray(sl), "xt": xT})
    nc = _get_nc(dtype_name)
    res = None
    last_err = None
    for attempt in range(3):
        try:
            res = run_bass_kernel_spmd(
                nc, in_maps, core_ids=list(range(N_CORES)), trace=trace
            )
            break
        except Exception as e:  # wedged-device NRT errors recover on retry
            last_err = e
            import time as _time

            _time.sleep(2.0)
    if res is None:
        raise last_err
    LAST_EXEC_NS = res.exec_time_ns
    LAST_RESULTS = res
    parts = np.stack(
        [res.results[k]["out"].reshape(BM).astype(np.float64) for k in range(N_CORES)]
    )
    return parts.sum(axis=0)


def kernel(pred_embs, pred_ps, gt_labels, weight):
    pred_embs = np.asarray(pred_embs, dtype=np.float32)
    pred_ps = np.asarray(pred_ps, dtype=np.float32)
    gt_labels = np.asarray(gt_labels)
    weight = np.asarray(weight, dtype=np.float32)

    trace = os.environ.get("KERNEL_TRACE", "0") == "1"

    # --- host marshalling: l2 normalize both operands (f32, like the ref) ---
    x = pred_embs.reshape(BM, D)
    xn = _l2n(x)                                           # [128, 192]
    wn = _l2n(weight)                                      # [200000, 192]

    # --- device: all-class sum of exp(30*cos - 30), sharded over 8 cores ---
    sum_full = _device_sumexp(xn, wn, DTYPE, trace=trace)  # [128] f64
    sum_full = sum_full.reshape(B, M)

    # --- host: labels, mirroring jax.lax.top_k(gt_labels, S_SPK)[1]
    # (indices of the S_SPK largest entries; ties broken by ascending index)
    labels = np.argsort(-gt_labels, axis=1, kind="stable")[:, :S_SPK]

    # --- host: exact cos at label columns (128 rows of W) ---
    xn64 = xn.reshape(B, M, D).astype(np.float64)
    wl = _l2n(weight[labels]).astype(np.float64)           # [B, S, D]
    cos_l = np.einsum("bmd,bsd->bms", xn64, wl)            # [B, M, S]

    sin_l = np.sqrt(np.clip(1.0 - cos_l**2, 0.0, 1.0))
    phi_l = cos_l * COS_M - sin_l * SIN_M
    phi_l = np.where(cos_l > TH, phi_l, cos_l - MM)

    # logsumexp with the label column replaced by phi (shift = SCALE)
    adj = (
        sum_full[:, :, None]
        - np.exp(SCALE * cos_l - SCALE)
        + np.exp(SCALE * phi_l - SCALE)
    )
    lse = SCALE + np.log(adj)                              # [B, M, S]
    ce = lse - SCALE * phi_l
    C = np.swapaxes(ce, 1, 2)                              # [B, S, M]

    # Hungarian on 4x4 via brute force over 24 permutations
    import itertools

    perms = np.array(list(itertools.permutations(range(S_SPK))), np.int64)  # [P,S]
    pc = C[:, np.arange(S_SPK)[None, :], perms].sum(-1)    # [B, P]
    best = np.argmin(pc, axis=1)
    col = perms[best]                                      # [B, S]

    matched = C[np.arange(B)[:, None], np.arange(S_SPK)[None, :], col]
    L_spk = matched.mean(axis=1)                           # [B]

    t_exist = np.zeros((B, M), np.float64)
    t_exist[np.arange(B)[:, None], col] = 1.0
    p = np.clip(pred_ps.astype(np.float64), EPS, 1.0 - EPS)
    L_exist = -(t_exist * np.log(p) + (1.0 - t_exist) * np.log(1.0 - p)).mean(axis=1)
    L_stop = -np.log(np.clip(pred_ps[:, -1].astype(np.float64), EPS, 1.0 - EPS))

    L_total = 0.01 * L_spk + ETA * L_exist + XI * L_stop
    return (
        np.float32(L_total.mean()),
        np.float32(L_spk.mean()),
        np.float32(L_exist.mean()),
        np.float32(L_stop.mean()),
    )
